# revision 6
# baseline (speedup 1.0000x reference)
"""Equivariant rotation conv for Trainium2, 8-core batch-parallel.

Computes: rotate a (128*8, 128, 3, 3) filter bank by 8 data-dependent angles
(bilinear resampling), run a 3x3 same-padded conv of x (16,128,128,128) with
all 8*128 rotated filters, then max over the 8 rotations -> (16,128,128,128).

Sharding: data-parallel over batch, 2 images per core; the filter bank and
rotation coefficients are replicated.  On device, per core:
  - the 9x9 bilinear mixing matrix per rotation (a pure function of the 8
    rot_alpha scalars, computed on host) is applied to the filter bank with
    DVE multiply-accumulate chains -> rotated lhsT tiles [Cin, O] per tap,
  - the conv runs as 9 shifted PE matmuls (K=Cin=128 partitions, N=512
    spatial) accumulated in PSUM, one PSUM bank per 4 output rows, 8
    output-channel chunks = 8 rotations,
  - a running elementwise max over the rotation chunks on DVE,
  - result rows DMA back to DRAM.
"""

import numpy as np


def _install_axon_hooks_shim():
    """Provide antenv.axon_hooks (NTFF profile hook) when the image's antenv
    lacks it, so run_bass_kernel_spmd(trace=True) works instead of crashing
    on import.  The hook drives NRT profiling via ctypes into the axon PJRT
    plugin, mirroring the boot-side installer."""
    import contextlib
    import ctypes
    import os
    import sys
    import types

    try:
        import antenv.axon_hooks  # noqa: F401

        return
    except ImportError:
        pass

    state = {"hook": None, "resolved": False}

    def _make_hook():
        so_path = os.environ.get("AXON_PJRT_SO", "/opt/axon/libaxon_pjrt.so")
        if not os.path.exists(so_path):
            return None
        lib = ctypes.CDLL(so_path)
        if not hasattr(lib, "axon_start_nrt_profile"):
            return None
        lib.axon_start_nrt_profile.argtypes = [
            ctypes.POINTER(ctypes.c_int64),
            ctypes.c_size_t,
        ]
        lib.axon_start_nrt_profile.restype = ctypes.c_int64
        lib.axon_stop_nrt_profile.argtypes = [ctypes.c_char_p]
        lib.axon_stop_nrt_profile.restype = ctypes.c_int64

        @contextlib.contextmanager
        def _hook(output_dir, device_ids):
            import jax

            jax.devices()
            if device_ids:
                ids = (ctypes.c_int64 * len(device_ids))(*device_ids)
                rc = lib.axon_start_nrt_profile(ids, len(device_ids))
            else:
                rc = lib.axon_start_nrt_profile(None, 0)
            if rc != 0:
                raise RuntimeError(f"axon_start_nrt_profile rc={rc}")
            try:
                yield
            finally:
                n = lib.axon_stop_nrt_profile(str(output_dir).encode())
                if n < 0:
                    raise RuntimeError(f"axon_stop_nrt_profile rc={n}")
                print(f"profile: {n} file(s) written to {output_dir}")

        return _hook

    mod = types.ModuleType("antenv.axon_hooks")

    def set_axon_ntff_profile_hook(h):
        state["hook"] = h
        state["resolved"] = True

    def get_axon_ntff_profile_hook():
        if not state["resolved"]:
            state["hook"] = _make_hook()
            state["resolved"] = True
        return state["hook"]

    mod.set_axon_ntff_profile_hook = set_axon_ntff_profile_hook
    mod.get_axon_ntff_profile_hook = get_axon_ntff_profile_hook
    sys.modules["antenv.axon_hooks"] = mod


_install_axon_hooks_shim()

import concourse.bass as bass
import concourse.mybir as mybir
from concourse import bacc
from concourse.bass_utils import run_bass_kernel_spmd
from concourse.tile import TileContext

F32 = mybir.dt.float32
F32R = mybir.dt.float32r
BF16 = mybir.dt.bfloat16

B, CIN, H, W = 16, 128, 128, 128
R, O, K = 8, 128, 3
NCORES = 8
BL = B // NCORES  # images per core
RB = 32           # output rows per block
NS = RB // 4      # psum subtiles (4 rows = 512 cols) per block
NBLK = H // RB

# "bf16": matmul operands bf16 (f32 accumulate).  "f32r": float32r operands.
MM_DTYPE = "bf16"

_TRACE = False
LAST_RESULTS = None
_NC_CACHE = {}


def _rot_mats(rot_alpha):
    """Per-rotation 9x9 bilinear resampling matrices, matching the reference
    F.grid_sample(align_corners=True, zeros) tap logic exactly.

    M[r, p, q]: coefficient of original tap q = (qy*3+qx) in rotated tap
    p = (py*3+px)."""
    M = np.zeros((R, 9, 9), np.float64)
    lin = np.linspace(-1.0, 1.0, K)
    for r in range(R):
        ang = float(rot_alpha[r]) * (np.pi / 4.0) * r
        c, s = np.cos(ang), np.sin(ang)
        for a in range(K):          # output row (gy = lin[a])
            for b in range(K):      # output col (gx = lin[b])
                gx, gy = lin[b], lin[a]
                xs = c * gx - s * gy
                ys = s * gx + c * gy
                ix = (xs + 1.0) * 0.5 * (K - 1)
                iy = (ys + 1.0) * 0.5 * (K - 1)
                x0 = int(np.floor(ix))
                y0 = int(np.floor(iy))
                wx, wy = ix - x0, iy - y0
                p = a * K + b
                for yi, xi, wt in (
                    (y0, x0, (1 - wy) * (1 - wx)),
                    (y0, x0 + 1, (1 - wy) * wx),
                    (y0 + 1, x0, wy * (1 - wx)),
                    (y0 + 1, x0 + 1, wy * wx),
                ):
                    if 0 <= yi < K and 0 <= xi < K:
                        M[r, p, yi * K + xi] += wt
    return M.astype(np.float32)


def _build(mm_dtype):
    use_bf16 = mm_dtype == "bf16"
    mm_dt = BF16 if use_bf16 else F32

    nc = bacc.Bacc(trn_type="TRN2")
    xs = nc.dram_tensor("xs", [BL, CIN, H, W], F32, kind="ExternalInput")
    # wl[r, i, :1152] = weights (q, o); wl[r, i, 1152:1233] = M[r] coefficients
    # (replicated across i) so each rotation needs exactly one input DMA.
    wl = nc.dram_tensor("wl", [R, CIN, 9 * O + 81], F32, kind="ExternalInput")
    y = nc.dram_tensor("y", [BL, O, H, W], F32, kind="ExternalOutput")

    with TileContext(nc) as tc:
        with (
            tc.tile_pool(name="consts", bufs=1) as cpool,
            tc.tile_pool(name="wsrc", bufs=1) as wpool,
            tc.tile_pool(name="wrot", bufs=1) as rpool,
            tc.tile_pool(name="rtmp", bufs=2) as tpool,
            tc.tile_pool(name="xio", bufs=2) as xpool,
            tc.tile_pool(name="xbfp", bufs=2) as xbpool,
            tc.tile_pool(name="accp", bufs=2) as apool,
            tc.tile_pool(name="psum", bufs=1, space="PSUM") as ppool,
        ):
            worig = []
            rotw = []
            for r in range(R):
                wsr = wpool.tile([128, 9 * O + 81], F32, name=f"wsr{r}", tag=f"wsr{r}")
                nc.sync.dma_start(out=wsr[:, :], in_=wl[r, :, :])
                worig.append(wsr)
                rw = rpool.tile([128, 9, O], mm_dt, name=f"rotw{r}", tag=f"rotw{r}")
                rotw.append(rw)

            def emit_rotate(r):
                # rotw[r][i, p, o] = sum_q M[r,p,q] * worig[r][i, q, o]
                if use_bf16:
                    rt = tpool.tile([128, 9, O], F32, name=f"rt{r}", tag="rt")
                else:
                    rt = rotw[r]
                wsr = worig[r]
                for p in range(9):
                    dst = rt[:, p, :]
                    base = 9 * O + p * 9
                    nc.vector.tensor_scalar_mul(
                        dst, wsr[:, 0 : O], wsr[:, base : base + 1]
                    )
                    for q in range(1, 9):
                        nc.vector.scalar_tensor_tensor(
                            dst,
                            wsr[:, q * O : (q + 1) * O],
                            wsr[:, base + q : base + q + 1],
                            dst,
                            mybir.AluOpType.mult,
                            mybir.AluOpType.add,
                        )
                if use_bf16:
                    nc.vector.tensor_copy(rotw[r][:, :, :], rt[:, :, :])

            # Rotate the first two banks up front; the rest are emitted
            # interleaved with block 0's chunks so DVE rotation overlaps PE
            # conv instead of serializing ahead of the first max ops.
            emit_rotate(0)
            emit_rotate(1)
            next_rot = [2]

            for b in range(BL):
                for blk in range(NBLK):
                    h0 = blk * RB
                    r0 = max(h0 - 1, 0)
                    r1 = min(h0 + RB + 1, H)
                    xst = xpool.tile([128, RB + 2, W + 2], F32, name="xst", tag="xst")
                    nc.gpsimd.memset(xst[:, :, :], 0.0)
                    nc.sync.dma_start(
                        out=xst[:, r0 - (h0 - 1) : r1 - (h0 - 1), 1 : W + 1],
                        in_=xs[b, :, r0:r1, :],
                    )
                    if use_bf16:
                        xmm = xbpool.tile(
                            [128, RB + 2, W + 2], BF16, name="xmm", tag="xmm"
                        )
                        nc.vector.tensor_copy(xmm[:, :, :], xst[:, :, :])
                    else:
                        xmm = xst
                    acc = apool.tile([128, RB, W], F32, name="acc", tag="acc")
                    for r in range(R):
                        pst = [
                            ppool.tile([128, 4, W], F32, name=f"ps{s}", tag=f"ps{s}")
                            for s in range(NS)
                        ]
                        for p in range(9):
                            ky, kx = divmod(p, 3)
                            lhsT = rotw[r][:, p, :]
                            if not use_bf16:
                                lhsT = lhsT.bitcast(F32R)
                            for s in range(NS):
                                rhs = xmm[:, 4 * s + ky : 4 * s + ky + 4, kx : kx + W]
                                if not use_bf16:
                                    rhs = rhs.bitcast(F32R)
                                nc.tensor.matmul(
                                    pst[s][:, :, :],
                                    lhsT,
                                    rhs,
                                    start=(p == 0),
                                    stop=(p == 8),
                                )
                        for s in range(NS):
                            if r == 0:
                                nc.vector.tensor_copy(
                                    acc[:, 4 * s : 4 * s + 4, :], pst[s][:, :, :]
                                )
                            else:
                                nc.vector.tensor_tensor(
                                    acc[:, 4 * s : 4 * s + 4, :],
                                    acc[:, 4 * s : 4 * s + 4, :],
                                    pst[s][:, :, :],
                                    mybir.AluOpType.max,
                                )
                        if b == 0 and blk == 0 and next_rot[0] < R:
                            emit_rotate(next_rot[0])
                            next_rot[0] += 1
                    nc.sync.dma_start(out=y[b, :, h0 : h0 + RB, :], in_=acc[:, :, :])
    nc.finalize()
    return nc


def _get_nc():
    if MM_DTYPE not in _NC_CACHE:
        _NC_CACHE[MM_DTYPE] = _build(MM_DTYPE)
    return _NC_CACHE[MM_DTYPE]


def kernel(x, weight, rot_alpha):
    global LAST_RESULTS
    x = np.ascontiguousarray(np.asarray(x, np.float32))
    weight = np.ascontiguousarray(np.asarray(weight, np.float32))
    rot_alpha = np.asarray(rot_alpha, np.float32)

    M = _rot_mats(rot_alpha)
    # wl[r, i, :1152] = weight[o*R + r, i, qy, qx] laid out (q, o);
    # wl[r, i, 1152:] = M[r] flattened (replicated across i).
    wq = weight.reshape(O, R, CIN, 9).transpose(1, 2, 3, 0).reshape(R, CIN, 9 * O)
    mrep = np.broadcast_to(M.reshape(R, 1, 81), (R, CIN, 81))
    wl = np.ascontiguousarray(np.concatenate([wq, mrep], axis=2), dtype=np.float32)

    nc = _get_nc()
    in_maps = [
        {"xs": np.ascontiguousarray(x[c * BL : (c + 1) * BL]), "wl": wl}
        for c in range(NCORES)
    ]
    res = run_bass_kernel_spmd(nc, in_maps, list(range(NCORES)), trace=_TRACE)
    LAST_RESULTS = res
    return np.concatenate([res.results[c]["y"] for c in range(NCORES)], axis=0)
